# revision 15
# baseline (speedup 1.0000x reference)
"""Trainium2 Bass kernel for MultiHeadAttention with RoPE.

Problem: B=2, L=2048, d_model=1024, 16 heads, d_k=64, fp32 in/out.

Sharding (8 cores): batch x head-group.  Core c owns batch c//4 and the 4
heads 4*(c%4)..4*(c%4)+3 (a 256-wide slice of the projection dims).  Each
core reads only its batch's q/k/v (transposed + bf16 on host), its 256-row
slice of Wq/Wk/Wv (pre-transposed) and the matching 256 columns of Wo.
The host sums the 4 partial outputs per batch and adds bo.

Per-core pipeline (all matmuls bf16, fp32 PSUM accumulation):
  1. QKV projections: [256 dims, 2048 tok], K=1024 in 8 chunks
  2. RoPE via partition-shifted DMA copy + 3 DVE ops; 1/sqrt(dk) and the
     rotate-half sign are folded into host-built cos/sin tables
  3. scores per head via SAME-HEAD block-diagonal packing: chunk c is
     diag(kh[:, 128c:128c+64], kh[:, 128c+64:128c+128]) against a
     partition-duplicated q rhs -> PSUM [128 contiguous kt, qt]
  4. exp on ScalarE (no max-subtract: scores ~ N(0,1)), bf16 out
  5. ctx per head: dense K=128 kt chunks, stationary vaug [128 kt, 65]
     whose 65th column is ones -> row 64 of the PSUM accumulator is the
     softmax denominator for free
  6. normalize: reciprocal + gpsimd partition broadcast + DVE multiply
  7. out_partial[tok, 1024] = ctx (stationary, K=256 in 2 passes) @ WoT

Scheduling: the exp stream on ScalarE is the pacing engine during
attention (~17.8us per head-block vs ~13.6us of PE work), so the second
head-pair's projections, the vaug transposes and the first 8 out-proj
tiles are emitted as fine-grained "fill" steps interleaved into the
attention chunk loops, keeping TensorE dense (no >3.4us idle gaps that
would re-throttle the HAM clock gate).
"""

import numpy as np
import ml_dtypes

import concourse.bass as bass
import concourse.mybir as mybir
import concourse.tile as tile
from concourse import bacc
from concourse.bass_utils import run_bass_kernel_spmd

BF = mybir.dt.bfloat16
F32 = mybir.dt.float32
AF = mybir.ActivationFunctionType
ALU = mybir.AluOpType

NCORES = 8
B = 2
L = 2048
D = 1024          # d_model
H = 16            # heads
DK = 64           # head dim
HPC = 4           # heads per core
PD = HPC * DK     # projection dims per core = 256
TOK = L           # tokens per core (one batch)
P = 128
NMT = PD // P     # matmul M-tiles per projection = 2

ROPE_BASE = 10000.0


def build_nc(debug_dumps=False):
    """Build the single-core Bass program (SPMD: same program, per-core data)."""
    from contextlib import ExitStack

    nc = bacc.Bacc("TRN2", target_bir_lowering=False, debug=False)
    dbg = {}
    if debug_dumps:
        for nm, shp, dt in [
            ("dbg_vaug", [P, 16 * 65], BF), ("dbg_qs2", [P, L], BF),
            ("dbg_kh2", [P, L], BF), ("dbg_vh", [P, L], BF),
            ("dbg_ex", [P, 1024], BF), ("dbg_cfull", [65, 1024], F32),
            ("dbg_rec", [1, 1024], F32), ("dbg_bcs", [DK, 1024], F32),
            ("dbg_ctx", [P, L], BF),
        ]:
            dbg[nm] = nc.dram_tensor(nm, shp, dt, kind="ExternalOutput").ap()

    # ---- DRAM I/O ----
    qT = nc.dram_tensor("qT", [D, TOK], BF, kind="ExternalInput").ap()
    kT = nc.dram_tensor("kT", [D, TOK], BF, kind="ExternalInput").ap()
    vT = nc.dram_tensor("vT", [D, TOK], BF, kind="ExternalInput").ap()
    wqT = nc.dram_tensor("wqT", [D, PD], BF, kind="ExternalInput").ap()
    wkT = nc.dram_tensor("wkT", [D, PD], BF, kind="ExternalInput").ap()
    wvT = nc.dram_tensor("wvT", [D, PD], BF, kind="ExternalInput").ap()
    woT = nc.dram_tensor("woT", [PD, D], BF, kind="ExternalInput").ap()
    bq_d = nc.dram_tensor("bq", [PD, 1], F32, kind="ExternalInput").ap()
    bk_d = nc.dram_tensor("bk", [PD, 1], F32, kind="ExternalInput").ap()
    bv_d = nc.dram_tensor("bv", [PD, 1], F32, kind="ExternalInput").ap()
    cos_q = nc.dram_tensor("cos_q", [P, L], BF, kind="ExternalInput").ap()
    sin_q = nc.dram_tensor("sin_q", [P, L], BF, kind="ExternalInput").ap()
    cos_k = nc.dram_tensor("cos_k", [P, L], BF, kind="ExternalInput").ap()
    sin_k = nc.dram_tensor("sin_k", [P, L], BF, kind="ExternalInput").ap()
    outp = nc.dram_tensor("outp", [TOK, D], BF, kind="ExternalOutput").ap()

    with tile.TileContext(nc) as tc, ExitStack() as ctx:
        const = ctx.enter_context(tc.tile_pool(name="const", bufs=1))
        persist = ctx.enter_context(tc.tile_pool(name="persist", bufs=1))
        stage = ctx.enter_context(tc.tile_pool(name="stage", bufs=12))
        raws = ctx.enter_context(tc.tile_pool(name="raws", bufs=2))
        rots = ctx.enter_context(tc.tile_pool(name="rots", bufs=2))
        expp = ctx.enter_context(tc.tile_pool(name="expp", bufs=5))
        outs = ctx.enter_context(tc.tile_pool(name="outs", bufs=3))
        smalls = ctx.enter_context(tc.tile_pool(name="smalls", bufs=2))
        scp = ctx.enter_context(tc.tile_pool(name="scp", bufs=2, space="PSUM"))
        fillp = ctx.enter_context(tc.tile_pool(name="fillp", bufs=1, space="PSUM"))
        cpp = ctx.enter_context(tc.tile_pool(name="cpp", bufs=1, space="PSUM"))

        # ---- constants into SBUF (phase order: q-proj consts first) ----
        def load_w(name, w_d):
            # [1024, 256] -> [128, 8, 256]
            w_sb = const.tile([P, 8 * PD], BF, name=name)
            nc.sync.dma_start(
                w_sb.rearrange("p (a m) -> p a m", a=8),
                w_d.rearrange("(a p) m -> p a m", p=P),
            )
            return w_sb

        def load_b(name, b_d):
            # [256, 1] -> [128, 2]
            b_sb = const.tile([P, NMT], F32, name=name)
            nc.sync.dma_start(
                b_sb.rearrange("p (a m) -> p a m", a=NMT),
                b_d.rearrange("(a p) m -> p a m", p=P),
            )
            return b_sb

        def load_c(name, t_d):
            t_sb = const.tile([P, L], BF, name=name)
            nc.sync.dma_start(t_sb[:], t_d[:])
            return t_sb

        wq_sb = load_w("wq_sb", wqT)
        bq_sb = load_b("bq_sb", bq_d)
        cq_sb = load_c("cq_sb", cos_q)
        sq_sb = load_c("sq_sb", sin_q)

        # persistent per-head attention operands
        qs2 = [persist.tile([P, L], BF, name=f"qs2_{h}") for h in range(HPC)]
        kh2 = [persist.tile([P, L], BF, name=f"kh2_{h}") for h in range(HPC)]
        vaug = [persist.tile([P, 16 * 65], BF, name=f"vaug_{h}")
                for h in range(HPC)]
        vh_sb = [persist.tile([P, L], BF, name=f"vh_{m}") for m in range(NMT)]
        ctx_sb = [persist.tile([P, L], BF, name=f"ctx_{m}") for m in range(NMT)]
        for t in kh2:
            nc.gpsimd.memset(t[:], 0.0)
        for t in vaug:
            nc.vector.memset(
                t.rearrange("p (c u) -> p c u", u=65)[:, :, 64:65], 1.0)

        ident = const.tile([P, P], BF)
        from concourse.masks import make_identity
        make_identity(nc, ident[:])
        ones_sb = const.tile([P, 1024], BF)
        nc.vector.memset(ones_sb[:], 1.0)
        # remaining const loads are emitted later (after q staging DMAs) so
        # the first projection matmuls are not queued behind them
        late_consts = {}

        def load_late_consts():
            late_consts["wk_sb"] = load_w("wk_sb", wkT)
            late_consts["bk_sb"] = load_b("bk_sb", bk_d)
            late_consts["ck_sb"] = load_c("ck_sb", cos_k)
            late_consts["sk_sb"] = load_c("sk_sb", sin_k)
            late_consts["wv_sb"] = load_w("wv_sb", wvT)
            late_consts["bv_sb"] = load_b("bv_sb", bv_d)
            wo = [const.tile([P, D], BF, name=f"wo_{m}") for m in range(NMT)]
            for m in range(NMT):
                nc.sync.dma_start(wo[m][:], woT[m * P:(m + 1) * P, :])
            late_consts["wo_sb"] = wo

        # ---------- emission-step generators ----------
        def gen_proj(x_d, w_sb, bias_sb, mt, kind, on_act, dma_eng=None):
            """Yield fine-grained steps projecting x -> M-tile mt.

            kind: 'q' | 'k' | 'v'.  on_act: evict on ScalarE (pre-phase)
            vs DVE scalar_tensor_tensor (mid-attention fill).  All tile
            allocations happen inside the yielded closures so pool slot
            rotation follows EMISSION order, not generator-build order.
            """
            st = {}
            w_r = w_sb.rearrange("p (a m) -> p a m", a=8)

            eng = dma_eng if dma_eng is not None else nc.sync

            def dma_all():
                st["xs"] = [stage.tile([P, L], BF, name="xstage", tag="stage")
                            for _ in range(8)]
                for kc in range(8):
                    eng.dma_start(st["xs"][kc][:],
                                  x_d[kc * P:(kc + 1) * P, :])
            yield dma_all

            for j in range(2):
                for kc in range(8):
                    def mm(kc=kc, j=j):
                        if kc == 0:
                            pool = scp if on_act else fillp
                            st["ps"] = pool.tile([P, 1024], F32,
                                                 name=f"pj{mt}{j}", tag="mm")
                        for nb in range(2):
                            c0 = j * 1024 + nb * 512
                            nc.tensor.matmul(
                                st["ps"][:, nb * 512:(nb + 1) * 512],
                                lhsT=w_r[:, kc, mt * P:(mt + 1) * P],
                                rhs=st["xs"][kc][:, c0:c0 + 512],
                                start=(kc == 0), stop=(kc == 7),
                                skip_group_check=True,
                            )
                    yield mm

                def evict(j=j):
                    if kind != "v" and j == 0:
                        st["raw"] = raws.tile([P, L], BF,
                                              name=f"raw{kind}{mt}", tag="raw")
                    dst = vh_sb[mt] if kind == "v" else st["raw"]
                    dsl = dst[:, j * 1024:(j + 1) * 1024]
                    if on_act:
                        nc.scalar.activation(dsl, st["ps"][:], AF.Identity,
                                             bias=bias_sb[:, mt:mt + 1])
                    else:
                        nc.vector.scalar_tensor_tensor(
                            dsl, st["ps"][:], bias_sb[:, mt:mt + 1],
                            ones_sb[:], ALU.add, ALU.mult)
                yield evict

            if kind == "v":
                return

            cos_sb, sin_sb = ((cq_sb, sq_sb) if kind == "q" else
                              (late_consts["ck_sb"], late_consts["sk_sb"]))

            def rope_dma():
                raw = st["raw"]
                rot = rots.tile([P, L], BF, name=f"rot{kind}{mt}", tag="rot")
                st["rot"] = rot
                for hb in range(2):
                    r0 = hb * DK
                    nc.sync.dma_start(rot[r0:r0 + 32, :],
                                      raw[r0 + 32:r0 + 64, :])
                    nc.sync.dma_start(rot[r0 + 32:r0 + 64, :],
                                      raw[r0:r0 + 32, :])
            yield rope_dma

            def rope_mul1():
                nc.vector.tensor_mul(st["raw"][:], st["raw"][:], cos_sb[:])
            yield rope_mul1

            def rope_mul2():
                nc.vector.tensor_mul(st["rot"][:], st["rot"][:], sin_sb[:])
                nc.vector.tensor_add(st["rot"][:], st["rot"][:], st["raw"][:])
            yield rope_mul2

            if kind == "q":
                for hl in range(2):
                    def bq_(hl=hl):
                        h = 2 * mt + hl
                        src = st["rot"][hl * DK:(hl + 1) * DK, :]
                        nc.vector.tensor_copy(qs2[h][0:DK, :], src)
                        nc.vector.tensor_copy(qs2[h][DK:P, :], src)
                    yield bq_
            else:
                for hl in range(2):
                    def bk_(hl=hl):
                        h = 2 * mt + hl
                        src = st["rot"][hl * DK:(hl + 1) * DK, :].rearrange(
                            "p (c g u) -> p c g u", g=2, u=DK)
                        dst = kh2[h].rearrange("p (c g u) -> p c g u",
                                               g=2, u=DK)
                        nc.vector.tensor_copy(dst[0:DK, :, 0, :],
                                              src[:, :, 0, :])
                        nc.vector.tensor_copy(dst[DK:P, :, 1, :],
                                              src[:, :, 1, :])
                    yield bk_

        def gen_vaug(mt, pool):
            """PE-transpose vh [dims, kt] -> vaug [kt, dims] (+ones col)."""
            dsts = [vaug[2 * mt + hl].rearrange("p (c u) -> p c u", u=65)
                    for hl in range(2)]
            for c in range(16):
                def tr(c=c):
                    pt = pool.tile([P, P], BF, name="pt", tag="mm")
                    nc.tensor.transpose(
                        pt[:], vh_sb[mt][:, c * P:(c + 1) * P], ident[:])
                    for hl in range(2):
                        nc.vector.tensor_copy(
                            dsts[hl][:, c, 0:DK], pt[:, hl * DK:(hl + 1) * DK])
                yield tr

        def gen_oproj(tiles, pool, alt_act=False):
            """Out-projection for the given token tiles."""
            for tb in tiles:
                def mm(tb=tb):
                    t0 = tb * P
                    po = pool.tile([P, D], F32, name="po", tag="mm")
                    for mt in range(NMT):
                        for nb in range(2):
                            nc.tensor.matmul(
                                po[:, nb * 512:(nb + 1) * 512],
                                lhsT=ctx_sb[mt][:, t0:t0 + P],
                                rhs=late_consts["wo_sb"][mt][
                                    :, nb * 512:(nb + 1) * 512],
                                start=(mt == 0), stop=(mt == NMT - 1),
                                skip_group_check=True,
                            )
                    ob = outs.tile([P, D], BF, name="ob", tag="out")
                    if alt_act and tb % 2 == 1:
                        nc.scalar.activation(ob[:], po[:], AF.Identity)
                    else:
                        nc.vector.tensor_copy(ob[:], po[:])
                    nc.sync.dma_start(outp[t0:t0 + P, :], ob[:])
                yield mm

        # ---------- attention with interleaved fill steps ----------
        fills = []

        def pump(n):
            for _ in range(n):
                if fills:
                    fills.pop(0)()

        def attention(h, j):
            """Head h, query block j (1024 tokens): scores/exp/ctx."""
            qs = qs2[h][:, j * 1024:(j + 1) * 1024]
            cp = cpp.tile([65, 1024], F32, name="cp", tag="cp")
            ex_prev = None
            for c in range(17):
                ex_cur = None
                if c < 16:
                    sc = scp.tile([P, 1024], F32, name="sc", tag="mm")
                    for nb in range(2):
                        nc.tensor.matmul(
                            sc[:, nb * 512:(nb + 1) * 512],
                            lhsT=kh2[h][:, c * P:(c + 1) * P],
                            rhs=qs[:, nb * 512:(nb + 1) * 512],
                            start=True, stop=True, skip_group_check=True,
                        )
                    ex_cur = expp.tile([P, 1024], BF, name="ex", tag="exp")
                    nc.scalar.activation(ex_cur[:], sc[:], AF.Exp)
                    if debug_dumps and h == 0 and j == 0 and c == 0:
                        nc.sync.dma_start(dbg["dbg_ex"][:], ex_cur[:])
                if c >= 1:
                    cpv = c - 1
                    for nb in range(2):
                        sl = slice(nb * 512, (nb + 1) * 512)
                        nc.tensor.matmul(
                            cp[:, sl],
                            lhsT=vaug[h][:, cpv * 65:(cpv + 1) * 65],
                            rhs=ex_prev[:, sl],
                            start=(cpv == 0), stop=(cpv == 15),
                            skip_group_check=True,
                        )
                pump(2)
                ex_prev = ex_cur
            # normalize: ctx_sb rows = ctx_unnorm * broadcast(1/den);
            # two pipelined 512-col half-chains so the last unit's exposed
            # latency is short and the PE never idles long enough to cool
            mt, hl = h // 2, h % 2
            cfull = smalls.tile([65, 1024], F32, name="cfull", tag="cfull")
            den = smalls.tile([1, 1024], F32, name="den", tag="den")
            rec = smalls.tile([1, 1024], F32, name="rec", tag="rec")
            bcs = smalls.tile([DK, 1024], F32, name="bcs", tag="bcs")
            for hf in range(2):
                sl = slice(hf * 512, (hf + 1) * 512)
                nc.vector.tensor_copy(cfull[:, sl], cp[:, sl])
                nc.sync.dma_start(den[:, sl], cfull[64:65, sl])
                nc.vector.reciprocal_approx_fast(rec[:, sl], den[:, sl])
                nc.gpsimd.partition_broadcast(bcs[:, sl], rec[:, sl],
                                              channels=DK)
                nc.vector.tensor_mul(
                    ctx_sb[mt][hl * DK:(hl + 1) * DK,
                               j * 1024 + hf * 512:j * 1024 + (hf + 1) * 512],
                    cfull[0:DK, sl], bcs[:, sl])
            if debug_dumps and h == 0 and j == 0:
                nc.sync.dma_start(dbg["dbg_cfull"][:], cfull[:])
                nc.sync.dma_start(dbg["dbg_rec"][:], rec[:])
                nc.sync.dma_start(dbg["dbg_bcs"][:], bcs[:])

        # ---------- program ----------
        # pre-phase: first head-pair projections + v/vaug, PE-dense.
        # The q staging DMAs go out first; the remaining const loads are
        # emitted right after them so the first matmuls aren't starved.
        qsteps = list(gen_proj(qT, wq_sb, bq_sb, 0, "q", on_act=True))
        qsteps[0]()
        load_late_consts()
        for step in qsteps[1:]:
            step()
        for step in gen_proj(kT, late_consts["wk_sb"], late_consts["bk_sb"],
                             0, "k", on_act=True, dma_eng=nc.gpsimd):
            step()
        for step in gen_proj(vT, late_consts["wv_sb"], late_consts["bv_sb"],
                             0, "v", on_act=True):
            step()
        for step in gen_vaug(0, scp):
            step()

        # fills consumed inside the attention chunk loops (order matters:
        # vh[1]/vaug[1] before h2 ctx, qs2/kh2 h2,h3 before h2 scores)
        fills.extend(gen_proj(vT, late_consts["wv_sb"], late_consts["bv_sb"],
                              1, "v", on_act=False, dma_eng=nc.gpsimd))
        fills.extend(gen_vaug(1, fillp))
        fills.extend(gen_proj(qT, wq_sb, bq_sb, 1, "q", on_act=False))
        fills.extend(gen_proj(kT, late_consts["wk_sb"], late_consts["bk_sb"],
                              1, "k", on_act=False, dma_eng=nc.gpsimd))

        attention(0, 0)
        attention(1, 0)
        attention(0, 1)
        attention(1, 1)
        # drain any remaining projection fills before h2 needs them
        pump(len(fills))
        attention(2, 0)
        attention(3, 0)
        fills.extend(gen_oproj(range(0, 8), fillp))
        attention(2, 1)
        attention(3, 1)
        pump(len(fills))
        for step in gen_oproj(range(8, 16), scp, alt_act=True):
            step()

        if debug_dumps:
            nc.sync.dma_start(dbg["dbg_vaug"][:], vaug[0][:])
            nc.sync.dma_start(dbg["dbg_qs2"][:], qs2[0][:])
            nc.sync.dma_start(dbg["dbg_kh2"][:], kh2[0][:])
            nc.sync.dma_start(dbg["dbg_vh"][:], vh_sb[0][:])
            nc.sync.dma_start(dbg["dbg_ctx"][:], ctx_sb[0][:])

    return nc


def _rope_tables():
    """Host-built RoPE tables, transposed to [d, t], 2 heads stacked.

    sin is sign-folded for the rotate-half convention; q tables carry the
    1/sqrt(dk) attention scale.
    """
    inv_freq = 1.0 / (ROPE_BASE ** (np.arange(0, DK, 2, dtype=np.float64) / DK))
    t = np.arange(L, dtype=np.float64)
    ang = np.outer(t, inv_freq)               # [L, 32]
    emb = np.concatenate([ang, ang], axis=1)  # [L, 64]
    cos = np.cos(emb).T.astype(np.float32)    # [64, L]
    sin = np.sin(emb).T.astype(np.float32)
    sin_folded = sin.copy()
    sin_folded[:32] *= -1.0
    scale = 1.0 / np.sqrt(DK)
    cos2 = np.concatenate([cos, cos], axis=0)                # [128, L]
    sin2 = np.concatenate([sin_folded, sin_folded], axis=0)  # [128, L]
    bf = ml_dtypes.bfloat16
    return (
        (cos2 * scale).astype(bf), (sin2 * scale).astype(bf),
        cos2.astype(bf), sin2.astype(bf),
    )


_NC_CACHE = {}


def _get_nc():
    if "nc" not in _NC_CACHE:
        nc = build_nc()
        nc.finalize()
        _NC_CACHE["nc"] = nc
    return _NC_CACHE["nc"]


def make_in_maps(q, k, v, Wq, bq, Wk, bk, Wv, bv, Wo, bo):
    bf = ml_dtypes.bfloat16
    cos_q, sin_q, cos_k, sin_k = _rope_tables()
    xT = {}
    for b in range(B):
        xT[("q", b)] = np.ascontiguousarray(np.asarray(q)[b].T).astype(bf)
        xT[("k", b)] = np.ascontiguousarray(np.asarray(k)[b].T).astype(bf)
        xT[("v", b)] = np.ascontiguousarray(np.asarray(v)[b].T).astype(bf)
    in_maps = []
    for c in range(NCORES):
        b, g = c // 4, c % 4
        hs = slice(g * PD, (g + 1) * PD)
        in_maps.append({
            "qT": xT[("q", b)], "kT": xT[("k", b)], "vT": xT[("v", b)],
            "wqT": np.ascontiguousarray(np.asarray(Wq)[hs, :].T).astype(bf),
            "wkT": np.ascontiguousarray(np.asarray(Wk)[hs, :].T).astype(bf),
            "wvT": np.ascontiguousarray(np.asarray(Wv)[hs, :].T).astype(bf),
            "woT": np.ascontiguousarray(np.asarray(Wo)[:, hs].T).astype(bf),
            "bq": np.asarray(bq[hs], np.float32).reshape(PD, 1),
            "bk": np.asarray(bk[hs], np.float32).reshape(PD, 1),
            "bv": np.asarray(bv[hs], np.float32).reshape(PD, 1),
            "cos_q": cos_q, "sin_q": sin_q, "cos_k": cos_k, "sin_k": sin_k,
        })
    return in_maps


def kernel(q, k, v, Wq, bq, Wk, bk, Wv, bv, Wo, bo):
    assert q.shape == (B, L, D) and k.shape == (B, L, D) and v.shape == (B, L, D)
    in_maps = make_in_maps(q, k, v, Wq, bq, Wk, bk, Wv, bv, Wo, bo)
    nc = _get_nc()
    res = run_bass_kernel_spmd(nc, in_maps, list(range(NCORES)))
    out = np.zeros((B, TOK, D), np.float64)
    for c, r in enumerate(res.results):
        out[c // 4] += r["outp"].astype(np.float64)
    out += np.asarray(bo, np.float64)[None, None, :]
    return out.astype(np.float32)
